# revision 11
# baseline (speedup 1.0000x reference)
"""Tensor-parallel MultiHeadAttention + LayerNorm kernel for 8 TRN2 NeuronCores.

Problem (all fp32):
    x [2048, 1024], 16 heads, dk=64
    q/k/v = x @ w_{q,k,v}(per head) + bias
    out = LayerNorm(concat_heads(softmax(q k^T / 8) v) @ wo + o_bias + x)

Sharding: tensor-parallel over heads. Core r owns heads (2r, 2r+1):
  - computes qT/kT (transposed, [dk, tokens]) and v for its two heads
  - scoresT[j, i] = k_j . q_i / 8 per 128-row j-chunk; exp on ACT
    (no max subtraction: |logits| <= ~16, exp fits fp32/bf16 comfortably)
  - attention output accumulated as [dk+1, tokens] via a ones-column in v,
    giving the softmax denominator in the extra row
  - FOUR AllToAll phases (64 tokens/shard each) pipelined against compute;
    each core ends with the full 1024-channel concat^T for its 256 tokens
  - output projection (full wo) + residual + o_bias, LayerNorm on the shard

v2 perf structure (prev best 194us):
  - PE is power-throttled to ~0.63 of 2.4GHz when all 8 cores run; cost is
    ~0.625ns per moving column -> ~124us floor for the ~198K columns here.
    So the design goal is zero PE idle, not fewer instructions.
  - QKV emitted k-tile-outer (q+k interleaved per xT tile) so the PE chases
    the 11us xT DMA instead of waiting for all 8 tiles
  - softmax denominator: Pool-copy row 64 -> K=1 bf16 ones matmul broadcast
    into PSUM base-0 -> reciprocal_approx_fast; no DRAM round trip
  - v_bias folded into o_bias host-side ((1 (x) vb) @ wo is constant)
  - one exp per (phase, jc) covering both heads' [128, 1024] scores
  - LN sqrt deferred until after the last exp (single ACT table switch)

Matmul dtypes (end-to-end relmax ~1.5e-3 of scale on baseline):
  - QKV / scores / projection: fp16 inputs; exp values + AV: bf16
  - accumulation fp32 in PSUM; softmax 1/s, LayerNorm, residual fp32
  - denominator broadcast travels through bf16 (adds ~1e-3 worst case)

Self-contained: hardcodes shapes; does not read anything from the problem dir.
"""

import os
import sys

for _p in ("/opt/trn_rl_repo", "/root/.axon_site/_ro/trn_rl_repo"):
    if os.path.isdir(_p) and _p not in sys.path:
        sys.path.insert(0, _p)

import numpy as np

import concourse.bass as bass
import concourse.tile as tile
from concourse import bacc, mybir
from concourse.bass_utils import run_bass_kernel_spmd



F32 = mybir.dt.float32
F16 = mybir.dt.float16
BF16 = mybir.dt.bfloat16
AF = mybir.ActivationFunctionType
ALU = mybir.AluOpType

N = 2048          # tokens
DM = 1024         # d_model
H = 16            # heads
DK = 64           # head dim
NCORES = 8
HPC = H // NCORES  # heads per core = 2
SH = N // NCORES   # token shard per core = 256
KT = DM // 128     # k-tiles over d_model = 8
NPH = 4            # A2A phases
PT = SH // NPH     # tokens per shard per phase = 64
EPS = 1e-5
SCALE = 1.0 / 8.0  # 1/sqrt(dk)

_CACHE = {}


def build_program(dbg=False):
    nc = bacc.Bacc("TRN2", target_bir_lowering=False, debug=False,
                   num_devices=NCORES)

    def din(name, shape, dt=F32):
        return nc.dram_tensor(name, list(shape), dt, kind="ExternalInput")

    xT = din("xT", (DM, N), F16)
    # weight layout: [partition 128, KT*128] with block k = w[128k:128(k+1), :]
    wq = din("wq", (128, KT * 2 * DK), F16)
    wk = din("wk", (128, KT * 2 * DK), F16)
    wv = din("wv", (128, KT * 2 * DK), F16)
    biases = din("biases", (2 * DK, 2))  # columns: q bias, k bias
    wo = din("wo", (DM, DM), F16)
    resid = din("resid", (SH, DM), F16)  # x + o_bias + (1 (x) v_bias) @ wo
    ident = din("ident", (128, 128), BF16)
    out_sh = nc.dram_tensor("out_sh", [SH, DM], F32, kind="ExternalOutput")

    # one collective per token phase: phase p exchanges tokens
    # {256j+64p .. 256j+64p+63} for every shard j
    a2a_in = [nc.dram_tensor(f"a2a_in{p}", [DM, PT], F16) for p in range(NPH)]
    a2a_out = [nc.dram_tensor(f"a2a_out{p}", [DM, PT], F16) for p in range(NPH)]

    with tile.TileContext(nc) as tc:
        with (
            tc.tile_pool(name="consts", bufs=1) as consts,
            tc.tile_pool(name="xt", bufs=KT) as xtp,
            tc.tile_pool(name="wop", bufs=KT) as wop,
            tc.tile_pool(name="wqkv", bufs=1) as wqkvp,
            tc.tile_pool(name="big", bufs=1) as bigp,
            tc.tile_pool(name="vv", bufs=1) as vvp,
            tc.tile_pool(name="ex", bufs=2) as exp_pool,
            tc.tile_pool(name="small", bufs=2) as smallp,
            tc.tile_pool(name="ln", bufs=2) as lnp,
            # One PSUM pool, 8 banks as tags:
            #   sA, sB: [128,1024] (2 banks each) - QKV accum / scores rotation
            #   b4..b7: [*,512] (1 bank each)    - k-proj, po, bcast, proj
            tc.tile_pool(name="ps", bufs=1, space="PSUM") as psp,
        ):
            # ---------------- constants / small loads ----------------
            b_sb = consts.tile([128, 2], F32, tag="biases")
            nc.scalar.dma_start(b_sb[:], biases[:])
            qb_sb, kb_sb = b_sb[:, 0:1], b_sb[:, 1:2]
            # gpsimd SW-DGE casts if the host could not provide bf16
            ident_sb = consts.tile([128, 128], BF16, tag="ident")
            nc.gpsimd.dma_start(ident_sb[:], ident[:])
            ones_bf = consts.tile([1, 64], BF16, tag="ones_bf")
            nc.vector.memset(ones_bf[:], 1.0)
            eps_sb = consts.tile([128, 1], F32, tag="eps")
            nc.vector.memset(eps_sb[:], EPS)
            # preload the exp table set while input DMAs run
            warm = consts.tile([128, 1], F32, tag="warm")
            nc.scalar.activation(warm[:], eps_sb[:], AF.Exp)

            # weights host-prearranged as [128, KT, 128] (contiguous DMA)
            w_sb = {}
            for name, dram in (("q", wq), ("k", wk), ("v", wv)):
                t = wqkvp.tile([128, KT, 2 * DK], F16, tag=f"w{name}")
                nc.gpsimd.dma_start(
                    t[:], dram.ap().rearrange("p (k m) -> p k m", k=KT))
                w_sb[name] = t

            # ---------------- main input loads ----------------
            xt_sb = []
            for k in range(KT):
                t = xtp.tile([128, N], F16, tag="xt", name=f"xt{k}")
                nc.sync.dma_start(t[:], xT[128 * k:128 * (k + 1), :])
                xt_sb.append(t)
            # wo / resid on the scalar queue (separate SBUF, no tag sharing)
            wo_sb = []
            for k in range(KT):
                t = wop.tile([128, DM], F16, tag="wo", name=f"wo{k}")
                nc.scalar.dma_start(t[:], wo[128 * k:128 * (k + 1), :])
                wo_sb.append(t)
            resid_sb = bigp.tile([128, 2, DM], F16, tag="resid")
            for m in range(2):
                nc.scalar.dma_start(resid_sb[:, m, :],
                                    resid[128 * m:128 * (m + 1), :])

            # ---------------- QKV projections (transposed layout) --------
            # qT/kT: [128, N]; rows 0:64 = head0 [dk], rows 64:128 = head1.
            # k-tile-outer so each xT tile is consumed as it lands: per tile,
            # 4 q chunks (into sA/sB half-slices) + 4 k chunks (b4..b7).
            qT = bigp.tile([128, N], F16, tag="qT")
            kT = bigp.tile([128, N], F16, tag="kT")
            vT = bigp.tile([128, N], BF16, tag="vT")

            q_ps = [psp.tile([128, 1024], F32, tag=t, name=f"q_ps{i}")
                    for i, t in enumerate(("sA", "sB"))]
            k_ps = [psp.tile([128, 512], F32, tag=f"b{4 + c}", name=f"k_ps{c}")
                    for c in range(4)]
            for k in range(KT):
                st, sp = (k == 0), (k == KT - 1)
                for c in range(4):
                    nc.tensor.matmul(
                        q_ps[c // 2][:, 512 * (c % 2):512 * (c % 2 + 1)],
                        w_sb["q"][:, k, :],
                        xt_sb[k][:, 512 * c:512 * (c + 1)],
                        start=st, stop=sp)
                for c in range(4):
                    nc.tensor.matmul(
                        k_ps[c][:], w_sb["k"][:, k, :],
                        xt_sb[k][:, 512 * c:512 * (c + 1)],
                        start=st, stop=sp)
            for c in range(4):
                dst = slice(512 * c, 512 * (c + 1))
                nc.vector.tensor_scalar_add(
                    qT[:, dst], q_ps[c // 2][:, 512 * (c % 2):512 * (c % 2 + 1)],
                    qb_sb[:])
                nc.vector.tensor_scalar_add(kT[:, dst], k_ps[c][:], kb_sb[:])

            v_ps = [psp.tile([128, 1024], F32, tag=t, name=f"v_ps{i}")
                    for i, t in enumerate(("sA", "sB"))]
            for k in range(KT):
                for c in range(4):
                    nc.tensor.matmul(
                        v_ps[c // 2][:, 512 * (c % 2):512 * (c % 2 + 1)],
                        w_sb["v"][:, k, :],
                        xt_sb[k][:, 512 * c:512 * (c + 1)],
                        start=(k == 0), stop=(k == KT - 1))
            for c in range(4):
                nc.vector.tensor_copy(
                    vT[:, 512 * c:512 * (c + 1)],
                    v_ps[c // 2][:, 512 * (c % 2):512 * (c % 2 + 1)])

            # ---------------- v transpose to [tokens, dk] + ones column ---
            # vv chunk c: [128 tokens, 130] = [v_h0 | 1 | v_h1 | 1], bf16
            vvbig = vvp.tile([128, 16, 130], BF16, tag="vvbig")
            nc.vector.memset(vvbig[:, :, 64:65], 1.0)
            nc.vector.memset(vvbig[:, :, 129:130], 1.0)
            for c in range(16):
                pt = psp.tile([128, 128], BF16, tag=f"b{4 + c % 4}",
                              name=f"tr{c}")
                nc.tensor.transpose(pt[:], vT[:, 128 * c:128 * (c + 1)],
                                    ident_sb[:])
                nc.vector.tensor_copy(vvbig[:, c, 0:64], pt[:, 0:64])
                nc.vector.tensor_copy(vvbig[:, c, 65:129], pt[:, 64:128])
            vv = [vvbig[:, c, :] for c in range(16)]

            # ---------------- attention, 4 phases ----------------
            # concatT rows 64h:64h+64 = normalized head-h output (channels),
            # fp16, in PHASE-BLOCK order: column 512p+64j+b <-> global token
            # 256j+64p+b, so every normalization write is contiguous and each
            # A2A phase stages with a single rearranged DMA.
            concatT = bigp.tile([128, N], F16, tag="concatT")
            qTr = qT[:].rearrange("q (j t b) -> q j t b", t=NPH, b=PT)

            # ag[m]: concat^T channel blocks for proj chunk m (phases 2m,2m+1)
            ag = [bigp.tile([128, KT, 128], F16, tag=f"ag{m}", name=f"ag{m}")
                  for m in range(2)]

            def attn_phase(phase, after_scores0=None):
                """Scores/exp/AV for one phase. `after_scores0` emits the
                PREVIOUS phase's normalize+A2A between this phase's first
                scores and first AV: the Pool srow copy and the bc matmul
                overlap PE/ACT work, and the po tags (b4/b5) are read by the
                old normalize before this phase's first AV rewrites them."""
                po = [psp.tile([65, 512], F32, tag=f"b{4 + h}",
                               name=f"po{h}_{phase}")
                      for h in range(HPC)]
                for jc in range(16):
                    ps_s = psp.tile([128, 1024], F32, tag="sA" if jc % 2 == 0
                                    else "sB", name=f"sc_{phase}_{jc}")
                    for h in range(HPC):
                        nc.tensor.matmul(
                            ps_s[:, 512 * h:512 * (h + 1)],
                            kT[64 * h:64 * (h + 1), 128 * jc:128 * (jc + 1)],
                            qTr[64 * h:64 * (h + 1), :, phase, :],
                            start=True, stop=True,
                            tile_position=(64 * h, 0))
                    if jc == 0 and after_scores0 is not None:
                        after_scores0()
                    ex = exp_pool.tile([128, 1024], BF16,
                                       tag=f"ex{jc % 2}", name=f"ex_{phase}_{jc}")
                    nc.scalar.activation(ex[:], ps_s[:], AF.Exp, scale=SCALE)
                    for h in range(HPC):
                        nc.tensor.matmul(
                            po[h][:], vv[jc][:, 65 * h:65 * (h + 1)],
                            ex[:, 512 * h:512 * (h + 1)],
                            start=(jc == 0), stop=(jc == 15))
                return po

            def norm_a2a(phase, po):
                # normalize: row 64 of po[h] is the softmax denominator.
                # Pool-copy it to a base-0 bf16 row, broadcast across 64
                # partitions with a K=1 ones matmul into PSUM, reciprocal
                # there, then one fp32 multiply into concatT. No DRAM trip.
                for h in range(HPC):
                    srow = smallp.tile([1, 512], BF16, tag=f"srow{h}",
                                       name=f"srow{h}_{phase}")
                    nc.vector.tensor_copy(srow[:], po[h][64:65, :])
                    bc = psp.tile([64, 512], F32, tag="b6",
                                  name=f"bc{h}_{phase}")
                    nc.tensor.matmul(bc[:], ones_bf[:], srow[:],
                                     start=True, stop=True)
                    inv_sb = smallp.tile([64, 512], F32, tag=f"inv{h}",
                                         name=f"inv{h}_{phase}")
                    nc.vector.reciprocal_approx_fast(inv_sb[:], bc[:])
                    nc.vector.tensor_mul(
                        concatT[64 * h:64 * (h + 1),
                                512 * phase:512 * (phase + 1)],
                        po[h][0:64, :], inv_sb[:])

                # stage + exchange this phase's tokens in ONE rearranged DMA
                nc.sync.dma_start(
                    a2a_in[phase].ap().rearrange("(j q) t -> q j t", q=128),
                    concatT[:, 512 * phase:512 * (phase + 1)]
                    .rearrange("q (j t) -> q j t", j=NCORES))
                nc.gpsimd.collective_compute(
                    "AllToAll", ALU.bypass,
                    replica_groups=[list(range(NCORES))],
                    ins=[a2a_in[phase].ap()], outs=[a2a_out[phase].ap()])
                m, half = phase // 2, phase % 2
                nc.sync.dma_start(
                    ag[m][:, :, PT * half:PT * (half + 1)],
                    a2a_out[phase].ap().rearrange("(k q) t -> q k t", q=128))

            def proj_chunk(m):
                # token chunk m of my shard = phases 2m, 2m+1
                y = lnp.tile([128, DM], F32, tag=f"y{m}", name=f"y{m}")
                for s2 in range(2):
                    pp = psp.tile([128, 512], F32, tag="b7",
                                  name=f"proj{m}_{s2}")
                    for k in range(KT):
                        nc.tensor.matmul(
                            pp[:], ag[m][:, k, :],
                            wo_sb[k][:, 512 * s2:512 * (s2 + 1)],
                            start=(k == 0), stop=(k == KT - 1))
                    nc.vector.tensor_add(
                        y[:, 512 * s2:512 * (s2 + 1)], pp[:],
                        resid_sb[:, m, 512 * s2:512 * (s2 + 1)])
                stats = lnp.tile([128, 2, 6], F32, tag=f"stats{m}")
                for g in range(2):
                    nc.vector.bn_stats(stats[:, g, :],
                                       y[:, 512 * g:512 * (g + 1)])
                mv = lnp.tile([128, 2], F32, tag=f"mv{m}")
                nc.vector.bn_aggr(mv[:], stats[:])
                return y, mv

            def ln_finish(m, y, mv):
                rstd = lnp.tile([128, 1], F32, tag=f"rstd{m}")
                nc.scalar.activation(rstd[:], mv[:, 1:2], AF.Sqrt,
                                     bias=eps_sb[:])
                nc.vector.reciprocal(rstd[:], rstd[:])
                yo = lnp.tile([128, DM], F32, tag=f"yo{m}")
                nc.vector.tensor_scalar(
                    yo[:], y[:], scalar1=mv[:, 0:1], scalar2=rstd[:],
                    op0=ALU.subtract, op1=ALU.mult)
                eng = nc.scalar if m == 0 else nc.sync
                eng.dma_start(out_sh[128 * m:128 * (m + 1), :], yo[:])

            po0 = attn_phase(0)
            po1 = attn_phase(1, after_scores0=lambda: norm_a2a(0, po0))
            po2 = attn_phase(2, after_scores0=lambda: norm_a2a(1, po1))
            po3 = attn_phase(3, after_scores0=lambda: norm_a2a(2, po2))
            norm_a2a(3, po3)
            # proj0 fills the PE during the A2A3 wire time (ag0 long since
            # landed); proj1 runs as soon as reload3 lands.
            y0, mv0 = proj_chunk(0)
            ln_finish(0, y0, mv0)  # sqrt: single ACT table switch after exps
            y1, mv1 = proj_chunk(1)
            ln_finish(1, y1, mv1)

    nc.compile()
    return nc


def get_program():
    if "nc" not in _CACHE:
        _CACHE["nc"] = build_program()
    return _CACHE["nc"]


def _wprep(w3, h0, h1):
    """[1024, 128] head-pair weight -> [128, KT*128] fp16: block k along the
    free dim = rows 128k:128(k+1) of the weight (contiguous device DMA)."""
    wc = np.concatenate([w3[h0], w3[h1]], axis=1)  # [1024, 128]
    wk_ = wc.reshape(KT, 128, 2 * DK).transpose(1, 0, 2).reshape(128, KT * 2 * DK)
    return np.ascontiguousarray(wk_.astype(np.float16))


def make_in_maps(x, wq, q_bias, wk, k_bias, wv, v_bias, wo, o_bias):
    x = np.ascontiguousarray(np.asarray(x, dtype=np.float32))
    wq3 = np.asarray(wq, dtype=np.float32).reshape(H, DM, DK)
    wk3 = np.asarray(wk, dtype=np.float32).reshape(H, DM, DK)
    wv3 = np.asarray(wv, dtype=np.float32).reshape(H, DM, DK)
    q_bias = np.asarray(q_bias, dtype=np.float32)
    k_bias = np.asarray(k_bias, dtype=np.float32)
    v_bias = np.asarray(v_bias, dtype=np.float32)
    wo32 = np.asarray(wo, dtype=np.float32)
    wo16 = np.ascontiguousarray(wo32.astype(np.float16))
    o_bias = np.asarray(o_bias, dtype=np.float32)

    xT = np.ascontiguousarray(x.T.astype(np.float16))
    # fold v_bias through the output projection: concat row is
    # (softmax .. v) + vb per head; (1 (x) vb) @ wo is a constant row.
    vb_concat = v_bias.reshape(DM)  # [h, dk] -> concat channel order
    resid_base = x + (o_bias + vb_concat @ wo32)[None, :]
    ident = np.eye(128, dtype=np.float32)
    try:
        import ml_dtypes
        ident = ident.astype(ml_dtypes.bfloat16)
    except ImportError:
        pass
    in_maps = []
    for r in range(NCORES):
        h0, h1 = 2 * r, 2 * r + 1
        in_maps.append({
            "xT": xT,
            "wq": _wprep(wq3, h0, h1),
            "wk": _wprep(wk3, h0, h1),
            "wv": _wprep(wv3, h0, h1),
            "biases": np.ascontiguousarray(np.stack([
                np.concatenate([q_bias[h0], q_bias[h1]]),
                np.concatenate([k_bias[h0], k_bias[h1]])], axis=1)),
            "wo": wo16,
            "resid": np.ascontiguousarray(
                resid_base[SH * r:SH * (r + 1)].astype(np.float16)),
            "ident": ident,
        })
    return in_maps


def run_device(in_maps, **kwargs):
    nc = get_program()
    return run_bass_kernel_spmd(nc, in_maps, core_ids=list(range(NCORES)),
                                **kwargs)


def kernel(x, wq, q_bias, wk, k_bias, wv, v_bias, wo, o_bias, alpha, beta,
           n, d_model, h):
    assert int(n) == N and int(d_model) == DM and int(h) == H
    in_maps = make_in_maps(x, wq, q_bias, wk, k_bias, wv, v_bias, wo, o_bias)
    res = run_device(in_maps)
    out = np.concatenate([res.results[r]["out_sh"] for r in range(NCORES)],
                         axis=0)
    alpha = np.asarray(alpha, dtype=np.float32)
    beta = np.asarray(beta, dtype=np.float32)
    # device computes (y-mu)*rstd; alpha/beta are ones/zeros per the spec,
    # but apply them if they ever are not
    if not (np.all(alpha == 1.0) and np.all(beta == 0.0)):
        out = out * alpha[None, :] + beta[None, :]
    return np.ascontiguousarray(out.astype(np.float32))


# revision 13
# speedup vs baseline: 1.2000x; 1.2000x over previous
"""Tensor-parallel MultiHeadAttention + LayerNorm kernel for 8 TRN2 NeuronCores.

Problem (all fp32):
    x [2048, 1024], 16 heads, dk=64
    q/k/v = x @ w_{q,k,v}(per head) + bias
    out = LayerNorm(concat_heads(softmax(q k^T / 8) v) @ wo + o_bias + x)

Sharding: tensor-parallel over heads. Core r owns heads (2r, 2r+1):
  - computes qT/kT (transposed, [dk, tokens]) and v for its two heads
  - scoresT[j, i] = k_j . q_i / 8 per 128-row j-chunk; exp on ACT
    (no max subtraction: |logits| <= ~16, exp fits fp32/bf16 comfortably)
  - attention output accumulated as [dk+1, tokens] via a ones-column in v,
    giving the softmax denominator in the extra row
  - FOUR AllToAll phases (64 tokens/shard each) pipelined against compute;
    each core ends with the full 1024-channel concat^T for its 256 tokens
  - output projection (full wo) + residual + o_bias, LayerNorm on the shard

v2 perf structure (prev best 194us):
  - PE is power-throttled to ~0.63 of 2.4GHz when all 8 cores run; cost is
    ~0.625ns per moving column -> ~124us floor for the ~198K columns here.
    So the design goal is zero PE idle, not fewer instructions.
  - QKV emitted k-tile-outer (q+k interleaved per xT tile) so the PE chases
    the 11us xT DMA instead of waiting for all 8 tiles
  - softmax denominator: Pool-copy row 64 -> K=1 bf16 ones matmul broadcast
    into PSUM base-0 -> reciprocal_approx_fast; no DRAM round trip
  - v_bias folded into o_bias host-side ((1 (x) vb) @ wo is constant)
  - one exp per (phase, jc) covering both heads' [128, 1024] scores
  - LN sqrt deferred until after the last exp (single ACT table switch)

Matmul dtypes (end-to-end relmax ~1.5e-3 of scale on baseline):
  - QKV / scores / projection: fp16 inputs; exp values + AV: bf16
  - accumulation fp32 in PSUM; softmax 1/s, LayerNorm, residual fp32
  - denominator broadcast travels through bf16 (adds ~1e-3 worst case)

Self-contained: hardcodes shapes; does not read anything from the problem dir.
"""

import os
import sys

for _p in ("/opt/trn_rl_repo", "/root/.axon_site/_ro/trn_rl_repo"):
    if os.path.isdir(_p) and _p not in sys.path:
        sys.path.insert(0, _p)

import numpy as np

import concourse.bass as bass
import concourse.tile as tile
from concourse import bacc, mybir
from concourse.bass_utils import run_bass_kernel_spmd



F32 = mybir.dt.float32
F16 = mybir.dt.float16
BF16 = mybir.dt.bfloat16
AF = mybir.ActivationFunctionType
ALU = mybir.AluOpType

N = 2048          # tokens
DM = 1024         # d_model
H = 16            # heads
DK = 64           # head dim
NCORES = 8
HPC = H // NCORES  # heads per core = 2
SH = N // NCORES   # token shard per core = 256
KT = DM // 128     # k-tiles over d_model = 8
NPH = 4            # A2A phases
PT = SH // NPH     # tokens per shard per phase = 64
EPS = 1e-5
SCALE = 1.0 / 8.0  # 1/sqrt(dk)

_CACHE = {}


def build_program(dbg=False):
    nc = bacc.Bacc("TRN2", target_bir_lowering=False, debug=False,
                   num_devices=NCORES)

    def din(name, shape, dt=F32):
        return nc.dram_tensor(name, list(shape), dt, kind="ExternalInput")

    xT = din("xT", (DM, N), F16)
    # weight layout: [partition 128, KT*128] with block k = w[128k:128(k+1), :]
    wq = din("wq", (128, KT * 2 * DK), F16)
    wk = din("wk", (128, KT * 2 * DK), F16)
    wv = din("wv", (128, KT * 2 * DK), F16)
    biases = din("biases", (2 * DK, 2))  # columns: q bias, k bias
    wo = din("wo", (DM, DM), F16)
    resid = din("resid", (SH, DM), F16)  # x + o_bias + (1 (x) v_bias) @ wo
    ident = din("ident", (128, 128), BF16)
    out_sh = nc.dram_tensor("out_sh", [SH, DM], F32, kind="ExternalOutput")

    # one collective per PAIR of sub-phases (m=0,1): exchanges tokens
    # {256j+128m .. 256j+128m+127} for every shard j. Collectives on this
    # fabric cost ~8-20us each regardless of size, so fewer+bigger wins.
    a2a_in = [nc.dram_tensor(f"a2a_in{m}", [DM, 2 * PT], F16) for m in range(2)]
    a2a_out = [nc.dram_tensor(f"a2a_out{m}", [DM, 2 * PT], F16) for m in range(2)]

    with tile.TileContext(nc) as tc:
        with (
            tc.tile_pool(name="consts", bufs=1) as consts,
            tc.tile_pool(name="xt", bufs=KT) as xtp,
            tc.tile_pool(name="wop", bufs=KT) as wop,
            tc.tile_pool(name="wqkv", bufs=1) as wqkvp,
            tc.tile_pool(name="big", bufs=1) as bigp,
            tc.tile_pool(name="vv", bufs=1) as vvp,
            tc.tile_pool(name="ex", bufs=2) as exp_pool,
            tc.tile_pool(name="small", bufs=2) as smallp,
            tc.tile_pool(name="ln", bufs=2) as lnp,
            # One PSUM pool, 8 banks as tags:
            #   sA, sB: [128,1024] (2 banks each) - QKV accum / scores rotation
            #   b4..b7: [*,512] (1 bank each)    - k-proj, po, bcast, proj
            tc.tile_pool(name="ps", bufs=1, space="PSUM") as psp,
        ):
            # ---------------- constants / small loads ----------------
            b_sb = consts.tile([128, 2], F32, tag="biases")
            nc.scalar.dma_start(b_sb[:], biases[:])
            qb_sb, kb_sb = b_sb[:, 0:1], b_sb[:, 1:2]
            # gpsimd SW-DGE casts if the host could not provide bf16
            ident_sb = consts.tile([128, 128], BF16, tag="ident")
            nc.gpsimd.dma_start(ident_sb[:], ident[:])
            ones_bf = consts.tile([1, 64], BF16, tag="ones_bf")
            nc.vector.memset(ones_bf[:], 1.0)
            eps_sb = consts.tile([128, 1], F32, tag="eps")
            nc.vector.memset(eps_sb[:], EPS)
            # preload the exp table set while input DMAs run
            warm = consts.tile([128, 1], F32, tag="warm")
            nc.scalar.activation(warm[:], eps_sb[:], AF.Exp)

            # weights host-prearranged as [128, KT, 128] (contiguous DMA)
            # on the scalar HW queue AHEAD of wo/resid: wq gates the first
            # matmul (SW-DGE on gpsimd started ~14.3us and ran at ~100GB/s)
            w_sb = {}
            for name, dram in (("q", wq), ("k", wk), ("v", wv)):
                t = wqkvp.tile([128, KT, 2 * DK], F16, tag=f"w{name}")
                nc.scalar.dma_start(
                    t[:], dram.ap().rearrange("p (k m) -> p k m", k=KT))
                w_sb[name] = t

            # ---------------- main input loads ----------------
            xt_sb = []
            for k in range(KT):
                t = xtp.tile([128, N], F16, tag="xt", name=f"xt{k}")
                nc.sync.dma_start(t[:], xT[128 * k:128 * (k + 1), :])
                xt_sb.append(t)
            # wo / resid on the scalar queue (separate SBUF, no tag sharing)
            wo_sb = []
            for k in range(KT):
                t = wop.tile([128, DM], F16, tag="wo", name=f"wo{k}")
                nc.scalar.dma_start(t[:], wo[128 * k:128 * (k + 1), :])
                wo_sb.append(t)
            resid_sb = bigp.tile([128, 2, DM], F16, tag="resid")
            for m in range(2):
                nc.scalar.dma_start(resid_sb[:, m, :],
                                    resid[128 * m:128 * (m + 1), :])

            # ---------------- QKV projections (transposed layout) --------
            # qT/kT: [128, N]; rows 0:64 = head0 [dk], rows 64:128 = head1.
            # k-tile-outer so each xT tile is consumed as it lands: per tile,
            # 4 q chunks (into sA/sB half-slices) + 4 k chunks (b4..b7).
            qT = bigp.tile([128, N], F16, tag="qT")
            kT = bigp.tile([128, N], F16, tag="kT")
            vT = bigp.tile([128, N], BF16, tag="vT")

            q_ps = [psp.tile([128, 1024], F32, tag=t, name=f"q_ps{i}")
                    for i, t in enumerate(("sA", "sB"))]
            k_ps = [psp.tile([128, 512], F32, tag=f"b{4 + c}", name=f"k_ps{c}")
                    for c in range(4)]
            for k in range(KT):
                st, sp = (k == 0), (k == KT - 1)
                for c in range(4):
                    nc.tensor.matmul(
                        q_ps[c // 2][:, 512 * (c % 2):512 * (c % 2 + 1)],
                        w_sb["q"][:, k, :],
                        xt_sb[k][:, 512 * c:512 * (c + 1)],
                        start=st, stop=sp)
                for c in range(4):
                    nc.tensor.matmul(
                        k_ps[c][:], w_sb["k"][:, k, :],
                        xt_sb[k][:, 512 * c:512 * (c + 1)],
                        start=st, stop=sp)
            for c in range(4):
                dst = slice(512 * c, 512 * (c + 1))
                nc.vector.tensor_scalar_add(
                    qT[:, dst], q_ps[c // 2][:, 512 * (c % 2):512 * (c % 2 + 1)],
                    qb_sb[:])
                nc.vector.tensor_scalar_add(kT[:, dst], k_ps[c][:], kb_sb[:])

            v_ps = [psp.tile([128, 1024], F32, tag=t, name=f"v_ps{i}")
                    for i, t in enumerate(("sA", "sB"))]
            for k in range(KT):
                for c in range(4):
                    nc.tensor.matmul(
                        v_ps[c // 2][:, 512 * (c % 2):512 * (c % 2 + 1)],
                        w_sb["v"][:, k, :],
                        xt_sb[k][:, 512 * c:512 * (c + 1)],
                        start=(k == 0), stop=(k == KT - 1))
            for c in range(4):
                nc.vector.tensor_copy(
                    vT[:, 512 * c:512 * (c + 1)],
                    v_ps[c // 2][:, 512 * (c % 2):512 * (c % 2 + 1)])

            # ---------------- v transpose to [tokens, dk] + ones column ---
            # vv chunk c: [128 tokens, 130] = [v_h0 | 1 | v_h1 | 1], bf16
            vvbig = vvp.tile([128, 16, 130], BF16, tag="vvbig")
            nc.vector.memset(vvbig[:, :, 64:65], 1.0)
            nc.vector.memset(vvbig[:, :, 129:130], 1.0)
            for c in range(16):
                pt = psp.tile([128, 128], BF16, tag=f"b{4 + c % 4}",
                              name=f"tr{c}")
                nc.tensor.transpose(pt[:], vT[:, 128 * c:128 * (c + 1)],
                                    ident_sb[:])
                nc.vector.tensor_copy(vvbig[:, c, 0:64], pt[:, 0:64])
                nc.vector.tensor_copy(vvbig[:, c, 65:129], pt[:, 64:128])
            vv = [vvbig[:, c, :] for c in range(16)]

            # ---------------- attention, 4 phases ----------------
            # concatT rows 64h:64h+64 = normalized head-h output (channels),
            # fp16, in PHASE-BLOCK order: column 512p+64j+b <-> global token
            # 256j+64p+b, so every normalization write is contiguous and each
            # A2A phase stages with a single rearranged DMA.
            concatT = bigp.tile([128, N], F16, tag="concatT")
            qTr = qT[:].rearrange("q (j t b) -> q j t b", t=NPH, b=PT)

            # ag[m]: concat^T channel blocks for proj chunk m (phases 2m,2m+1)
            ag = [bigp.tile([128, KT, 128], F16, tag=f"ag{m}", name=f"ag{m}")
                  for m in range(2)]

            def attn_phase(phase, after_scores0=None):
                """Scores/exp/AV for one phase. `after_scores0` emits the
                PREVIOUS phase's normalize+A2A between this phase's first
                scores and first AV: the Pool srow copy and the bc matmul
                overlap PE/ACT work, and the po tags (b4/b5) are read by the
                old normalize before this phase's first AV rewrites them."""
                po = [psp.tile([65, 512], F32, tag=f"b{4 + h}",
                               name=f"po{h}_{phase}")
                      for h in range(HPC)]
                for jc in range(16):
                    ps_s = psp.tile([128, 1024], F32, tag="sA" if jc % 2 == 0
                                    else "sB", name=f"sc_{phase}_{jc}")
                    for h in range(HPC):
                        nc.tensor.matmul(
                            ps_s[:, 512 * h:512 * (h + 1)],
                            kT[64 * h:64 * (h + 1), 128 * jc:128 * (jc + 1)],
                            qTr[64 * h:64 * (h + 1), :, phase, :],
                            start=True, stop=True,
                            tile_position=(64 * h, 0))
                    if jc == 0 and after_scores0 is not None:
                        after_scores0()
                    ex = exp_pool.tile([128, 1024], BF16,
                                       tag=f"ex{jc % 2}", name=f"ex_{phase}_{jc}")
                    nc.scalar.activation(ex[:], ps_s[:], AF.Exp, scale=SCALE)
                    for h in range(HPC):
                        nc.tensor.matmul(
                            po[h][:], vv[jc][:, 65 * h:65 * (h + 1)],
                            ex[:, 512 * h:512 * (h + 1)],
                            start=(jc == 0), stop=(jc == 15))
                return po

            def norm(phase, po):
                # normalize: row 64 of po[h] is the softmax denominator.
                # Pool-copy it to a base-0 bf16 row, broadcast across 64
                # partitions with a K=1 ones matmul into PSUM, reciprocal
                # there, then one fp32 multiply into concatT. No DRAM trip.
                for h in range(HPC):
                    srow = smallp.tile([1, 512], BF16, tag=f"srow{h}",
                                       name=f"srow{h}_{phase}")
                    nc.vector.tensor_copy(srow[:], po[h][64:65, :])
                    bc = psp.tile([64, 512], F32, tag="b6",
                                  name=f"bc{h}_{phase}")
                    nc.tensor.matmul(bc[:], ones_bf[:], srow[:],
                                     start=True, stop=True)
                    inv_sb = smallp.tile([64, 512], F32, tag=f"inv{h}",
                                         name=f"inv{h}_{phase}")
                    nc.vector.reciprocal_approx_fast(inv_sb[:], bc[:])
                    nc.vector.tensor_mul(
                        concatT[64 * h:64 * (h + 1),
                                512 * phase:512 * (phase + 1)],
                        po[h][0:64, :], inv_sb[:])

                # stage this sub-phase into its half of the pair buffer:
                # a2a_in[m] col 64*half+b of row 128j+q <-> token 256j+64p+b
                m, half = phase // 2, phase % 2
                nc.sync.dma_start(
                    a2a_in[m].ap()[:, PT * half:PT * (half + 1)]
                    .rearrange("(j q) t -> q j t", q=128),
                    concatT[:, 512 * phase:512 * (phase + 1)]
                    .rearrange("q (j t) -> q j t", j=NCORES))

            def a2a_pair(m):
                nc.gpsimd.collective_compute(
                    "AllToAll", ALU.bypass,
                    replica_groups=[list(range(NCORES))],
                    ins=[a2a_in[m].ap()], outs=[a2a_out[m].ap()])
                nc.sync.dma_start(
                    ag[m][:],
                    a2a_out[m].ap().rearrange("(k q) t -> q k t", q=128))

            def proj_chunk(m):
                # token chunk m of my shard = phases 2m, 2m+1
                y = lnp.tile([128, DM], F32, tag=f"y{m}", name=f"y{m}")
                for s2 in range(2):
                    pp = psp.tile([128, 512], F32, tag="b7",
                                  name=f"proj{m}_{s2}")
                    for k in range(KT):
                        nc.tensor.matmul(
                            pp[:], ag[m][:, k, :],
                            wo_sb[k][:, 512 * s2:512 * (s2 + 1)],
                            start=(k == 0), stop=(k == KT - 1))
                    nc.vector.tensor_add(
                        y[:, 512 * s2:512 * (s2 + 1)], pp[:],
                        resid_sb[:, m, 512 * s2:512 * (s2 + 1)])
                stats = lnp.tile([128, 2, 6], F32, tag=f"stats{m}")
                for g in range(2):
                    nc.vector.bn_stats(stats[:, g, :],
                                       y[:, 512 * g:512 * (g + 1)])
                mv = lnp.tile([128, 2], F32, tag=f"mv{m}")
                nc.vector.bn_aggr(mv[:], stats[:])
                return y, mv

            def ln_finish(m, y, mv):
                rstd = lnp.tile([128, 1], F32, tag=f"rstd{m}")
                nc.scalar.activation(rstd[:], mv[:, 1:2], AF.Sqrt,
                                     bias=eps_sb[:])
                nc.vector.reciprocal(rstd[:], rstd[:])
                yo = lnp.tile([128, DM], F32, tag=f"yo{m}")
                nc.vector.tensor_scalar(
                    yo[:], y[:], scalar1=mv[:, 0:1], scalar2=rstd[:],
                    op0=ALU.subtract, op1=ALU.mult)
                eng = nc.scalar if m == 0 else nc.sync
                eng.dma_start(out_sh[128 * m:128 * (m + 1), :], yo[:])

            def cb1():
                norm(1, po1)
                a2a_pair(0)

            po0 = attn_phase(0)
            po1 = attn_phase(1, after_scores0=lambda: norm(0, po0))
            po2 = attn_phase(2, after_scores0=cb1)
            po3 = attn_phase(3, after_scores0=lambda: norm(2, po2))
            norm(3, po3)
            a2a_pair(1)
            # proj0 fills the PE during the A2A1 wire time (ag0 long since
            # landed); proj1 runs as soon as reload1 lands.
            y0, mv0 = proj_chunk(0)
            ln_finish(0, y0, mv0)  # sqrt: single ACT table switch after exps
            y1, mv1 = proj_chunk(1)
            ln_finish(1, y1, mv1)

    nc.compile()
    return nc


def get_program():
    if "nc" not in _CACHE:
        _CACHE["nc"] = build_program()
    return _CACHE["nc"]


def _wprep(w3, h0, h1):
    """[1024, 128] head-pair weight -> [128, KT*128] fp16: block k along the
    free dim = rows 128k:128(k+1) of the weight (contiguous device DMA)."""
    wc = np.concatenate([w3[h0], w3[h1]], axis=1)  # [1024, 128]
    wk_ = wc.reshape(KT, 128, 2 * DK).transpose(1, 0, 2).reshape(128, KT * 2 * DK)
    return np.ascontiguousarray(wk_.astype(np.float16))


def make_in_maps(x, wq, q_bias, wk, k_bias, wv, v_bias, wo, o_bias):
    x = np.ascontiguousarray(np.asarray(x, dtype=np.float32))
    wq3 = np.asarray(wq, dtype=np.float32).reshape(H, DM, DK)
    wk3 = np.asarray(wk, dtype=np.float32).reshape(H, DM, DK)
    wv3 = np.asarray(wv, dtype=np.float32).reshape(H, DM, DK)
    q_bias = np.asarray(q_bias, dtype=np.float32)
    k_bias = np.asarray(k_bias, dtype=np.float32)
    v_bias = np.asarray(v_bias, dtype=np.float32)
    wo32 = np.asarray(wo, dtype=np.float32)
    wo16 = np.ascontiguousarray(wo32.astype(np.float16))
    o_bias = np.asarray(o_bias, dtype=np.float32)

    xT = np.ascontiguousarray(x.T.astype(np.float16))
    # fold v_bias through the output projection: concat row is
    # (softmax .. v) + vb per head; (1 (x) vb) @ wo is a constant row.
    vb_concat = v_bias.reshape(DM)  # [h, dk] -> concat channel order
    resid_base = x + (o_bias + vb_concat @ wo32)[None, :]
    ident = np.eye(128, dtype=np.float32)
    try:
        import ml_dtypes
        ident = ident.astype(ml_dtypes.bfloat16)
    except ImportError:
        pass
    in_maps = []
    for r in range(NCORES):
        h0, h1 = 2 * r, 2 * r + 1
        in_maps.append({
            "xT": xT,
            "wq": _wprep(wq3, h0, h1),
            "wk": _wprep(wk3, h0, h1),
            "wv": _wprep(wv3, h0, h1),
            "biases": np.ascontiguousarray(np.stack([
                np.concatenate([q_bias[h0], q_bias[h1]]),
                np.concatenate([k_bias[h0], k_bias[h1]])], axis=1)),
            "wo": wo16,
            "resid": np.ascontiguousarray(
                resid_base[SH * r:SH * (r + 1)].astype(np.float16)),
            "ident": ident,
        })
    return in_maps


def run_device(in_maps, **kwargs):
    nc = get_program()
    return run_bass_kernel_spmd(nc, in_maps, core_ids=list(range(NCORES)),
                                **kwargs)


def kernel(x, wq, q_bias, wk, k_bias, wv, v_bias, wo, o_bias, alpha, beta,
           n, d_model, h):
    assert int(n) == N and int(d_model) == DM and int(h) == H
    in_maps = make_in_maps(x, wq, q_bias, wk, k_bias, wv, v_bias, wo, o_bias)
    res = run_device(in_maps)
    out = np.concatenate([res.results[r]["out_sh"] for r in range(NCORES)],
                         axis=0)
    alpha = np.asarray(alpha, dtype=np.float32)
    beta = np.asarray(beta, dtype=np.float32)
    # device computes (y-mu)*rstd; alpha/beta are ones/zeros per the spec,
    # but apply them if they ever are not
    if not (np.all(alpha == 1.0) and np.all(beta == 0.0)):
        out = out * alpha[None, :] + beta[None, :]
    return np.ascontiguousarray(out.astype(np.float32))
